# revision 25
# baseline (speedup 1.0000x reference)
"""Trainium2 Bass kernel for nn_MultiHeadCrossAttention (B=4, T=1024, E=1024, H=16).

Sharding: 8 fully independent shards, zero cross-core communication:
(output stream s, batch b) for s in {1,2}, b in 0..3. Stream-1 output
xo@Wout1 needs K,V from x and Q from y; stream-2 the reverse.

Per-core kernel (activations transposed, feature-on-partition):
  Preamble: V = A^T.T @ Wv^T (natural, with ones column per head for the
  rowsum trick); Q^T/K^T chunks 0,1.
  m-loop over 8 head pairs (hA=2m, hB=2m+1), 16 slots (jc, ic) each:
    S^T pair via two concurrent K=64 matmuls (tile_position row split)
    P = exp(S/8) on ACT into SBUF fp16 (A|B merged per slot)
    O'^T accumulation (M=65 incl. ones row -> rowsum) lagged one slot
    Q^T/K^T projection chunk m+2 interleaved (2 matmuls per slot)
    normalization of pair m-1 lagged: reciprocal_approx_fast + gpsimd
    partition_broadcast + fp16 multiply into ot
  Tail: Z^T = Wout^T.T @ O^T accumulated over head pairs, DMA out.
"""

import os
import sys

sys.path.insert(0, "/opt/trn_rl_repo")

import numpy as np
import ml_dtypes
from contextlib import ExitStack

import concourse.bass as bass
import concourse.mybir as mybir
import concourse.tile as tile
from concourse import bacc
from concourse import bass_utils

B, T, E, H = 4, 1024, 1024, 16
D = E // H            # 64
NC = E // 128         # 8 chunks of 128
N_CORES = 8

F32 = mybir.dt.float32
F16 = mybir.dt.float16

_NC_CACHE = {}
LAST_RESULTS = {}
_KDBG = os.environ.get("KDBG", "0") == "1"
_KDBG2 = os.environ.get("KDBG", "0") == "2"
_DBG_TILES = {}


def _build():
    nc = bacc.Bacc("TRN2", target_bir_lowering=False, debug=False,
                   enable_asserts=False, num_devices=N_CORES)
    a_t = nc.dram_tensor("a_t", (E, T), F16, kind="ExternalInput").ap()
    b_t = nc.dram_tensor("b_t", (E, T), F16, kind="ExternalInput").ap()
    wq_t = nc.dram_tensor("wq_t", (E, E), F16, kind="ExternalInput").ap()
    wk_t = nc.dram_tensor("wk_t", (E, E), F16, kind="ExternalInput").ap()
    wv_t = nc.dram_tensor("wv_t", (E, E), F16, kind="ExternalInput").ap()
    wout_t = nc.dram_tensor("wout_t", (E, E), F16, kind="ExternalInput").ap()
    z_t = nc.dram_tensor("z_t", (E, T), F32, kind="ExternalOutput").ap()

    EXP = mybir.ActivationFunctionType.Exp

    with tile.TileContext(nc) as tc, ExitStack() as ctx:
        persist = ctx.enter_context(tc.tile_pool(name="persist", bufs=1))
        qt = persist.tile([128, NC, T], F16, tag="qt")
        kt = persist.tile([128, NC, T], F16, tag="kt")
        v = persist.tile([128, NC, H * (D + 1)], F16, tag="v")
        ot = persist.tile([128, NC, T], F16, tag="ot")
        wo_sb = None if _KDBG else persist.tile([128, NC, E], F16, tag="wo", name="wo_sb")

        for mch in range(NC):
            nc.vector.memset(
                v[:, mch, :].rearrange("p (h x) -> p h x", x=D + 1)[:, :, D:D + 1],
                1.0)
        if _KDBG:
            _DBG_TILES["zd"] = persist.tile([128, 6, T], F32, tag="zd", name="zd")
            nc.vector.memset(_DBG_TILES["zd"][:], 0.0)

        acts = ctx.enter_context(tc.tile_pool(name="acts", bufs=1))
        at_sb = acts.tile([128, NC, T], F16, tag="at")
        bt_sb = acts.tile([128, NC, T], F16, tag="bt")
        wq_sb = acts.tile([128, NC, E], F16, tag="wq")
        wk_sb = acts.tile([128, NC, E], F16, tag="wk")

        # ---------------- Preamble: V proj + QK chunks 0,1 ----------------
        with tc.tile_pool(name="wvp", bufs=1) as wvp, \
             tc.tile_pool(name="pps", bufs=2, space="PSUM") as pps:
            wv_sb = wvp.tile([128, NC, E], F16, tag="wv")
            # DMA priority: (at, wv) pairs first so V proj starts ASAP
            for e in range(NC):
                nc.sync.dma_start(at_sb[:, e, :], a_t[e * 128:(e + 1) * 128, :])
                nc.sync.dma_start(wv_sb[:, e, :], wv_t[e * 128:(e + 1) * 128, :])
            for e in range(NC):
                nc.sync.dma_start(bt_sb[:, e, :], b_t[e * 128:(e + 1) * 128, :])
                nc.sync.dma_start(wq_sb[:, e, :], wq_t[e * 128:(e + 1) * 128, :])
            for e in range(NC):
                nc.sync.dma_start(wk_sb[:, e, :], wk_t[e * 128:(e + 1) * 128, :])
            for e in range(NC) if not _KDBG else ():
                nc.sync.dma_start(wo_sb[:, e, :], wout_t[e * 128:(e + 1) * 128, :])

            # V natural: out[j-chunk m][h*d] = sum_e at[e, j].T @ wv[e, hd]
            for mch in range(NC):
                ps = pps.tile([128, T], F32, tag="pp")
                for e in range(NC):
                    for ic in range(2):
                        nc.tensor.matmul(
                            ps[:, bass.ts(ic, 512)],
                            at_sb[:, e, bass.ts(mch, 128)],
                            wv_sb[:, e, bass.ts(ic, 512)],
                            start=(e == 0), stop=(e == NC - 1))
                with nc.allow_low_precision(reason="V fp16 feeds fp16 matmul"):
                    nc.vector.tensor_copy(
                        v[:, mch, :].rearrange("p (h x) -> p h x", x=D + 1)[:, :, 0:D],
                        ps[:].rearrange("p (h d) -> p h d", d=D))

            # Q^T/K^T chunks 0 and 1
            for ch in (0, 1):
                for (w_sb, act_sb, out_sb) in ((wq_sb, bt_sb, qt), (wk_sb, at_sb, kt)):
                    ps = pps.tile([128, T], F32, tag="pp")
                    for e in range(NC):
                        for ic in range(2):
                            nc.tensor.matmul(
                                ps[:, bass.ts(ic, 512)],
                                w_sb[:, e, bass.ts(ch, 128)],
                                act_sb[:, e, bass.ts(ic, 512)],
                                start=(e == 0), stop=(e == NC - 1))
                    with nc.allow_low_precision(reason="QK fp16 feeds fp16 matmul"):
                        nc.vector.tensor_copy(out_sb[:, ch, :], ps[:])

        # ---------------- m-loop: attention over 8 head pairs ----------------
        with tc.tile_pool(name="sps", bufs=2, space="PSUM") as sps_pool, \
             tc.tile_pool(name="ops", bufs=1, space="PSUM") as ops, \
             tc.tile_pool(name="pjp", bufs=1, space="PSUM") as pjp, \
             tc.tile_pool(name="ptp", bufs=6) as ptp, \
             tc.tile_pool(name="oup", bufs=2) as oup, \
             tc.tile_pool(name="nrm", bufs=2) as nrm:

            SLOTS = [(ic, jc) for ic in range(2) for jc in range(NC)]
            pending_o = None   # (pt_tile, jc, ic, ps_oA, ps_oB, hA, hB)
            pending_norm = None  # (m, ouA, ouB, rs)

            def issue_o(po):
                pt_prev, jc, psA, psB, hA, hB = po
                st = dict(start=(jc == 0), stop=(jc == NC - 1))
                nc.tensor.matmul(psA[:, :],
                                 v[:, jc, hA * (D + 1):(hA + 1) * (D + 1)],
                                 pt_prev[:, 0:512], **st)
                nc.tensor.matmul(psB[:, :],
                                 v[:, jc, hB * (D + 1):(hB + 1) * (D + 1)],
                                 pt_prev[:, 512:1024], **st)

            def issue_norm(pn):
                mm, ouA, ouB, rs2 = pn
                rr2 = nrm.tile([1, 2, T], F32, tag="rr2", bufs=1)
                nc.vector.reciprocal_approx_fast(rr2[:], rs2[:])
                rrh2 = nrm.tile([1, 2, T], F16, tag="rrh2", bufs=1)
                with nc.allow_low_precision(reason="recip feeds fp16 multiply"):
                    nc.vector.tensor_copy(rrh2[:], rr2[:])
                bcA = nrm.tile([64, T], F16, tag="bcA", bufs=1)
                bcB = nrm.tile([64, T], F16, tag="bcB", bufs=1)
                nc.gpsimd.partition_broadcast(bcA[:], rrh2[:, 0, :])
                nc.gpsimd.partition_broadcast(bcB[:], rrh2[:, 1, :])
                with nc.allow_low_precision(reason="O^T fp16 feeds fp16 out-proj"):
                    nc.vector.tensor_mul(ot[0:64, mm, :], ouA[:], bcA[:])
                    nc.vector.tensor_mul(ot[64:128, mm, :], ouB[:], bcB[:])
                if _KDBG and mm == 0:
                    zd = _DBG_TILES["zd"]
                    nc.vector.tensor_copy(zd[0:64, 0, :], ouB[:])
                    nc.vector.tensor_copy(zd[0:64, 1, :], bcB[:])
                    nc.vector.tensor_copy(zd[0:1, 2, :], rs2[:, 1, :])
                    nc.vector.tensor_copy(zd[32:33, 2, :], rr2[:, 1, :])
                    nc.vector.tensor_copy(zd[64:65, 2, :], rrh2[:, 1, :])

            def evac_half(psA, psB, ouA, ouB, rs2, ic):
                sl = bass.ts(ic, 512)
                with nc.allow_low_precision(reason="O' fp16 feeds fp16 multiply"):
                    nc.vector.tensor_copy(ouA[:, sl], psA[0:D, :])
                    nc.vector.tensor_copy(ouB[:, sl], psB[0:D, :])
                nc.vector.tensor_copy(rs2[:, 0, sl], psA[D:D + 1, :])
                nc.vector.tensor_copy(rs2[:, 1, sl], psB[D:D + 1, :])

            for m in range(NC):
                hA, hB = 2 * m, 2 * m + 1
                ps_oA = ops.tile([D + 1, 512], F32, tag="oA")
                ps_oB = ops.tile([D + 1, 512], F32, tag="oB")
                ouA = oup.tile([D, T], F16, tag="ouA")
                ouB = oup.tile([D, T], F16, tag="ouB")
                rs2 = nrm.tile([1, 2, T], F32, tag="rs2", bufs=1)
                if pending_norm is not None:
                    issue_norm(pending_norm)
                    pending_norm = None

                pj = None
                def issue_s(ic, jc):
                    sps = sps_pool.tile([128, 1024], F32, tag="s", name="sps")
                    nc.tensor.matmul(
                        sps[:, 0:512],
                        kt[0:64, m, bass.ts(jc, 128)],
                        qt[0:64, m, bass.ts(ic, 512)],
                        start=True, stop=True)
                    nc.tensor.matmul(
                        sps[:, 512:1024],
                        kt[64:128, m, bass.ts(jc, 128)],
                        qt[64:128, m, bass.ts(ic, 512)],
                        start=True, stop=True, tile_position=(64, 0))
                    return sps

                pending_s = issue_s(*SLOTS[0])
                for s, (ic, jc) in enumerate(SLOTS):
                    if s + 1 < len(SLOTS):
                        next_s = issue_s(*SLOTS[s + 1])
                    sps, pending_s = pending_s, next_s if s + 1 < len(SLOTS) else None
                    pt_t = ptp.tile([128, 1024], F16, tag="pt")
                    nc.scalar.activation(pt_t[:], sps[:], EXP, scale=0.125)
                    if _KDBG and m == 0 and s == 0:
                        nc.vector.tensor_copy(_DBG_TILES["zd"][:, 4, :], pt_t[:])

                    if pending_o is not None:
                        issue_o(pending_o)
                    if s == 8:
                        evac_half(ps_oA, ps_oB, ouA, ouB, rs2, 0)
                        ps_oA = ops.tile([D + 1, 512], F32, tag="oA")
                        ps_oB = ops.tile([D + 1, 512], F32, tag="oB")
                    pending_o = (pt_t, jc, ps_oA, ps_oB, hA, hB)

                    # interleaved Q^T/K^T projections:
                    # pairs 0-4: Q(m+2) slots 0-7, K(m+2) slots 8-15 (2 MM/slot)
                    # pair 5: Q(7) spread 1 MM/slot; pair 6: K(7) 1 MM/slot
                    if m < NC - 3:
                        ch = m + 2
                        w_p, a_p, o_p = (wq_sb, bt_sb, qt) if s < 8 else (wk_sb, at_sb, kt)
                        e = s % 8
                        if pj is None:
                            pj = pjp.tile([128, T], F32, tag="pj")
                        for icc in range(2):
                            nc.tensor.matmul(
                                pj[:, bass.ts(icc, 512)],
                                w_p[:, e, bass.ts(ch, 128)],
                                a_p[:, e, bass.ts(icc, 512)],
                                start=(e == 0), stop=(e == NC - 1))
                        if s in (7, 15):
                            with nc.allow_low_precision(reason="QK fp16"):
                                nc.vector.tensor_copy(o_p[:, ch, :], pj[:])
                            pj = None
                    elif m in (NC - 3, NC - 2) and s < 8:
                        ch = NC - 1
                        w_p, a_p, o_p = ((wq_sb, bt_sb, qt) if m == NC - 3
                                         else (wk_sb, at_sb, kt))
                        e = s
                        if pj is None:
                            pj = pjp.tile([128, T], F32, tag="pj")
                        for icc in range(2):
                            nc.tensor.matmul(
                                pj[:, bass.ts(icc, 512)],
                                w_p[:, e, bass.ts(ch, 128)],
                                a_p[:, e, bass.ts(icc, 512)],
                                start=(e == 0), stop=(e == NC - 1))
                        if s == 7:
                            with nc.allow_low_precision(reason="QK fp16"):
                                nc.vector.tensor_copy(o_p[:, ch, :], pj[:])
                            pj = None

                # flush last O slot of this pair, then evacuate half 1
                issue_o(pending_o)
                pending_o = None
                evac_half(ps_oA, ps_oB, ouA, ouB, rs2, 1)
                pending_norm = (m, ouA, ouB, rs2)

            issue_norm(pending_norm)
            pending_norm = None

        if _KDBG:
            with tc.tile_pool(name="zdbg2", bufs=1) as zp2:
                zd = _DBG_TILES["zd"]
                nc.vector.tensor_copy(zd[0:64, 3, :], ot[0:64, 0, :])
                nc.vector.tensor_copy(zd[64:128, 3, :], ot[64:128, 0, :])
                nc.vector.tensor_copy(zd[:, 5, :], qt[:, 2, :])
                for cc in range(6):
                    nc.sync.dma_start(z_t[cc * 128:(cc + 1) * 128, :],
                                      zd[:, cc, :])

        if _KDBG2:
            with tc.tile_pool(name="zdbg3", bufs=2) as zp3:
                for mm in range(NC):
                    zc = zp3.tile([128, T], F32, tag="zc", name="zc")
                    nc.vector.tensor_copy(zc[:], ot[:, mm, :])
                    nc.sync.dma_start(z_t[mm * 128:(mm + 1) * 128, :], zc[:])

        # ---------------- Z: out-projection ----------------
        if not _KDBG and not _KDBG2:
          with tc.tile_pool(name="zps", bufs=2, space="PSUM") as zps, \
             tc.tile_pool(name="zsb", bufs=2) as zsbp:
            for cc in range(NC):
                ps = zps.tile([128, T], F32, tag="z")
                for mm in range(NC):
                    for ic in range(2):
                        nc.tensor.matmul(
                            ps[:, bass.ts(ic, 512)],
                            wo_sb[:, mm, bass.ts(cc, 128)],
                            ot[:, mm, bass.ts(ic, 512)],
                            start=(mm == 0), stop=(mm == NC - 1))
                zsb = zsbp.tile([128, T], F32, tag="zsb")
                for ic in range(2):
                    nc.vector.tensor_copy(zsb[:, bass.ts(ic, 512)],
                                          ps[:, bass.ts(ic, 512)])
                    nc.sync.dma_start(
                        z_t[cc * 128:(cc + 1) * 128, ic * 512:(ic + 1) * 512],
                        zsb[:, bass.ts(ic, 512)])
    nc.compile()
    return nc


def _group_w(wqkv, k):
    """Rows of Wqkv (3E, E) for q/k/v (k=0/1/2), grouped head-major.

    Row index layout: r = di*(3H) + k*H + h  ->  grouped[h*D+di, :].
    """
    w = np.asarray(wqkv, dtype=np.float32).reshape(D, 3, H, E)[:, k]   # [di, h, e]
    return np.ascontiguousarray(w.transpose(1, 0, 2).reshape(E, E))    # [h*D+di, e]


def kernel(x, y, Wqkv1, Wqkv2, Wout1, Wout2):
    x = np.asarray(x, dtype=np.float32)
    y = np.asarray(y, dtype=np.float32)

    if "nc" not in _NC_CACHE:
        _NC_CACHE["nc"] = _build()
    nc = _NC_CACHE["nc"]

    wq1_t = np.ascontiguousarray(_group_w(Wqkv1, 0).T)
    wk1_t = np.ascontiguousarray(_group_w(Wqkv1, 1).T)
    wv1_t = np.ascontiguousarray(_group_w(Wqkv1, 2).T)
    wq2_t = np.ascontiguousarray(_group_w(Wqkv2, 0).T)
    wk2_t = np.ascontiguousarray(_group_w(Wqkv2, 1).T)
    wv2_t = np.ascontiguousarray(_group_w(Wqkv2, 2).T)
    wout1_t = np.ascontiguousarray(np.asarray(Wout1, dtype=np.float32).T)
    wout2_t = np.ascontiguousarray(np.asarray(Wout2, dtype=np.float32).T)

    in_maps = []
    for c in range(N_CORES):
        s, b = divmod(c, B)
        if s == 0:
            # stream-1 output: K,V from x via Wqkv1; Q from y via Wqkv2
            a_t, b_t = x[b].T, y[b].T
            wq, wk, wv, wo = wq2_t, wk1_t, wv1_t, wout1_t
        else:
            a_t, b_t = y[b].T, x[b].T
            wq, wk, wv, wo = wq1_t, wk2_t, wv2_t, wout2_t
        in_maps.append({
            "a_t": np.ascontiguousarray(a_t).astype(np.float16),
            "b_t": np.ascontiguousarray(b_t).astype(np.float16),
            "wq_t": wq.astype(np.float16), "wk_t": wk.astype(np.float16),
            "wv_t": wv.astype(np.float16), "wout_t": wo.astype(np.float16),
        })

    trace = os.environ.get("BASS_KERNEL_TRACE", "0") == "1"
    if trace:
        try:
            from antenv.axon_hooks import get_axon_ntff_profile_hook  # noqa: F401
        except ImportError:
            trace = False
    ncores = int(os.environ.get("KCORES", str(N_CORES)))
    r = bass_utils.run_bass_kernel_spmd(nc, in_maps[:ncores], core_ids=list(range(ncores)),
                                        trace=trace)
    LAST_RESULTS["exec_time_ns"] = r.exec_time_ns
    LAST_RESULTS["profile_json"] = r.profile_json

    out1 = np.stack([r.results[b]["z_t"].T for b in range(B)]).astype(np.float32)
    out2 = np.stack([r.results[B + b]["z_t"].T for b in range(B)]).astype(np.float32)
    return out1, out2


# revision 26
# speedup vs baseline: 1.0222x; 1.0222x over previous
"""Trainium2 Bass kernel for nn_MultiHeadCrossAttention (B=4, T=1024, E=1024, H=16).

Sharding: 8 fully independent shards, zero cross-core communication:
(output stream s, batch b) for s in {1,2}, b in 0..3. Stream-1 output
xo@Wout1 needs K,V from x and Q from y; stream-2 the reverse.

Per-core kernel (activations transposed, feature-on-partition):
  Preamble: V = A^T.T @ Wv^T (natural, with ones column per head for the
  rowsum trick); Q^T/K^T chunks 0,1.
  m-loop over 8 head pairs (hA=2m, hB=2m+1), 16 slots (jc, ic) each:
    S^T pair via two concurrent K=64 matmuls (tile_position row split)
    P = exp(S/8) on ACT into SBUF fp16 (A|B merged per slot)
    O'^T accumulation (M=65 incl. ones row -> rowsum) lagged one slot
    Q^T/K^T projection chunk m+2 interleaved (2 matmuls per slot)
    normalization of pair m-1 lagged: reciprocal_approx_fast + gpsimd
    partition_broadcast + fp16 multiply into ot
  Tail: Z^T = Wout^T.T @ O^T accumulated over head pairs, DMA out.
"""

import os
import sys

sys.path.insert(0, "/opt/trn_rl_repo")

import numpy as np
import ml_dtypes
from contextlib import ExitStack

import concourse.bass as bass
import concourse.mybir as mybir
import concourse.tile as tile
from concourse import bacc
from concourse import bass_utils

B, T, E, H = 4, 1024, 1024, 16
D = E // H            # 64
NC = E // 128         # 8 chunks of 128
N_CORES = 8

F32 = mybir.dt.float32
F16 = mybir.dt.float16

_NC_CACHE = {}
LAST_RESULTS = {}
_KDBG = os.environ.get("KDBG", "0") == "1"
_KDBG2 = os.environ.get("KDBG", "0") == "2"
_DBG_TILES = {}


def _build():
    nc = bacc.Bacc("TRN2", target_bir_lowering=False, debug=False,
                   enable_asserts=False, num_devices=N_CORES)
    a_t = nc.dram_tensor("a_t", (E, T), F16, kind="ExternalInput").ap()
    b_t = nc.dram_tensor("b_t", (E, T), F16, kind="ExternalInput").ap()
    wq_t = nc.dram_tensor("wq_t", (E, E), F16, kind="ExternalInput").ap()
    wk_t = nc.dram_tensor("wk_t", (E, E), F16, kind="ExternalInput").ap()
    wv_t = nc.dram_tensor("wv_t", (E, E), F16, kind="ExternalInput").ap()
    wout_t = nc.dram_tensor("wout_t", (E, E), F16, kind="ExternalInput").ap()
    z_t = nc.dram_tensor("z_t", (E, T), F32, kind="ExternalOutput").ap()

    EXP = mybir.ActivationFunctionType.Exp

    with tile.TileContext(nc) as tc, ExitStack() as ctx:
        persist = ctx.enter_context(tc.tile_pool(name="persist", bufs=1))
        qt = persist.tile([128, NC, T], F16, tag="qt")
        kt = persist.tile([128, NC, T], F16, tag="kt")
        v = persist.tile([128, NC, H * (D + 1)], F16, tag="v")
        ot = persist.tile([128, NC, T], F16, tag="ot")
        wo_sb = None if _KDBG else persist.tile([128, NC, E], F16, tag="wo", name="wo_sb")

        for mch in range(NC):
            nc.vector.memset(
                v[:, mch, :].rearrange("p (h x) -> p h x", x=D + 1)[:, :, D:D + 1],
                1.0)
        if _KDBG:
            _DBG_TILES["zd"] = persist.tile([128, 6, T], F32, tag="zd", name="zd")
            nc.vector.memset(_DBG_TILES["zd"][:], 0.0)

        acts = ctx.enter_context(tc.tile_pool(name="acts", bufs=1))
        at_sb = acts.tile([128, NC, T], F16, tag="at")
        bt_sb = acts.tile([128, NC, T], F16, tag="bt")
        wq_sb = acts.tile([128, NC, E], F16, tag="wq")
        wk_sb = acts.tile([128, NC, E], F16, tag="wk")

        # ---------------- Preamble: V proj + QK chunks 0,1 ----------------
        with tc.tile_pool(name="wvp", bufs=1) as wvp, \
             tc.tile_pool(name="pps", bufs=2, space="PSUM") as pps:
            wv_sb = wvp.tile([128, NC, E], F16, tag="wv")
            # DMA priority: (at, wv) pairs first so V proj starts ASAP
            for e in range(NC):
                nc.sync.dma_start(at_sb[:, e, :], a_t[e * 128:(e + 1) * 128, :])
                nc.sync.dma_start(wv_sb[:, e, :], wv_t[e * 128:(e + 1) * 128, :])
            for e in range(NC):
                nc.sync.dma_start(bt_sb[:, e, :], b_t[e * 128:(e + 1) * 128, :])
                nc.sync.dma_start(wq_sb[:, e, :], wq_t[e * 128:(e + 1) * 128, :])
            for e in range(NC):
                nc.sync.dma_start(wk_sb[:, e, :], wk_t[e * 128:(e + 1) * 128, :])
            for e in range(NC) if not _KDBG else ():
                nc.sync.dma_start(wo_sb[:, e, :], wout_t[e * 128:(e + 1) * 128, :])

            # V natural: out[j-chunk m][h*d] = sum_e at[e, j].T @ wv[e, hd]
            for mch in range(NC):
                ps = pps.tile([128, T], F32, tag="pp")
                for e in range(NC):
                    for ic in range(2):
                        nc.tensor.matmul(
                            ps[:, bass.ts(ic, 512)],
                            at_sb[:, e, bass.ts(mch, 128)],
                            wv_sb[:, e, bass.ts(ic, 512)],
                            start=(e == 0), stop=(e == NC - 1))
                with nc.allow_low_precision(reason="V fp16 feeds fp16 matmul"):
                    nc.vector.tensor_copy(
                        v[:, mch, :].rearrange("p (h x) -> p h x", x=D + 1)[:, :, 0:D],
                        ps[:].rearrange("p (h d) -> p h d", d=D))

            # Q^T/K^T chunks 0 and 1
            for ch in (0, 1):
                for (w_sb, act_sb, out_sb) in ((wq_sb, bt_sb, qt), (wk_sb, at_sb, kt)):
                    ps = pps.tile([128, T], F32, tag="pp")
                    for e in range(NC):
                        for ic in range(2):
                            nc.tensor.matmul(
                                ps[:, bass.ts(ic, 512)],
                                w_sb[:, e, bass.ts(ch, 128)],
                                act_sb[:, e, bass.ts(ic, 512)],
                                start=(e == 0), stop=(e == NC - 1))
                    with nc.allow_low_precision(reason="QK fp16 feeds fp16 matmul"):
                        nc.vector.tensor_copy(out_sb[:, ch, :], ps[:])

        # ---------------- m-loop: attention over 8 head pairs ----------------
        with tc.tile_pool(name="sps", bufs=2, space="PSUM") as sps_pool, \
             tc.tile_pool(name="ops", bufs=1, space="PSUM") as ops, \
             tc.tile_pool(name="pjp", bufs=1, space="PSUM") as pjp, \
             tc.tile_pool(name="ptp", bufs=6) as ptp, \
             tc.tile_pool(name="oup", bufs=2) as oup, \
             tc.tile_pool(name="nrm", bufs=2) as nrm:

            SLOTS = [(ic, jc) for ic in range(2) for jc in range(NC)]
            pending_o = None   # (pt_tile, jc, ic, ps_oA, ps_oB, hA, hB)
            pending_norm = None  # (m, ouA, ouB, rs)

            def issue_o(po):
                pt_prev, jc, psA, psB, hA, hB = po
                st = dict(start=(jc == 0), stop=(jc == NC - 1))
                nc.tensor.matmul(psA[:, :],
                                 v[:, jc, hA * (D + 1):(hA + 1) * (D + 1)],
                                 pt_prev[:, 0:512], **st)
                nc.tensor.matmul(psB[:, :],
                                 v[:, jc, hB * (D + 1):(hB + 1) * (D + 1)],
                                 pt_prev[:, 512:1024], **st)

            def issue_norm(pn):
                mm, ouA, ouB, rs2 = pn
                rr2 = nrm.tile([1, 2, T], F32, tag="rr2", bufs=1)
                nc.vector.reciprocal_approx_fast(rr2[:], rs2[:])
                rrh2 = nrm.tile([1, 2, T], F16, tag="rrh2", bufs=1)
                with nc.allow_low_precision(reason="recip feeds fp16 multiply"):
                    nc.vector.tensor_copy(rrh2[:], rr2[:])
                bcA = nrm.tile([64, T], F16, tag="bcA", bufs=1)
                bcB = nrm.tile([64, T], F16, tag="bcB", bufs=1)
                nc.gpsimd.partition_broadcast(bcA[:], rrh2[:, 0, :])
                nc.gpsimd.partition_broadcast(bcB[:], rrh2[:, 1, :])
                with nc.allow_low_precision(reason="O^T fp16 feeds fp16 out-proj"):
                    nc.vector.tensor_mul(ot[0:64, mm, :], ouA[:], bcA[:])
                    nc.vector.tensor_mul(ot[64:128, mm, :], ouB[:], bcB[:])
                if _KDBG and mm == 0:
                    zd = _DBG_TILES["zd"]
                    nc.vector.tensor_copy(zd[0:64, 0, :], ouB[:])
                    nc.vector.tensor_copy(zd[0:64, 1, :], bcB[:])
                    nc.vector.tensor_copy(zd[0:1, 2, :], rs2[:, 1, :])
                    nc.vector.tensor_copy(zd[32:33, 2, :], rr2[:, 1, :])
                    nc.vector.tensor_copy(zd[64:65, 2, :], rrh2[:, 1, :])

            def evac_half(psA, psB, ouA, ouB, rs2, ic):
                sl = bass.ts(ic, 512)
                with nc.allow_low_precision(reason="O' fp16 feeds fp16 multiply"):
                    nc.vector.tensor_copy(ouA[:, sl], psA[0:D, :])
                    nc.vector.tensor_copy(ouB[:, sl], psB[0:D, :])
                nc.vector.tensor_copy(rs2[:, 0, sl], psA[D:D + 1, :])
                nc.vector.tensor_copy(rs2[:, 1, sl], psB[D:D + 1, :])

            for m in range(NC):
                hA, hB = 2 * m, 2 * m + 1
                ps_oA = ops.tile([D + 1, 512], F32, tag="oA")
                ps_oB = ops.tile([D + 1, 512], F32, tag="oB")
                ouA = oup.tile([D, T], F16, tag="ouA")
                ouB = oup.tile([D, T], F16, tag="ouB")
                rs2 = nrm.tile([1, 2, T], F32, tag="rs2", bufs=1)
                if pending_norm is not None:
                    issue_norm(pending_norm)
                    pending_norm = None

                pj = None
                def issue_s(ic, jc):
                    sps = sps_pool.tile([128, 1024], F32, tag="s", name="sps")
                    nc.tensor.matmul(
                        sps[:, 0:512],
                        kt[0:64, m, bass.ts(jc, 128)],
                        qt[0:64, m, bass.ts(ic, 512)],
                        start=True, stop=True)
                    nc.tensor.matmul(
                        sps[:, 512:1024],
                        kt[64:128, m, bass.ts(jc, 128)],
                        qt[64:128, m, bass.ts(ic, 512)],
                        start=True, stop=True, tile_position=(64, 0))
                    return sps

                pending_s = issue_s(*SLOTS[0])
                for s, (ic, jc) in enumerate(SLOTS):
                    if s + 1 < len(SLOTS):
                        next_s = issue_s(*SLOTS[s + 1])
                    sps, pending_s = pending_s, next_s if s + 1 < len(SLOTS) else None
                    pt_t = ptp.tile([128, 1024], F16, tag="pt")
                    nc.scalar.activation(pt_t[:], sps[:], EXP, scale=0.125)
                    if _KDBG and m == 0 and s == 0:
                        nc.vector.tensor_copy(_DBG_TILES["zd"][:, 4, :], pt_t[:])

                    if pending_o is not None:
                        issue_o(pending_o)
                    if s == 8:
                        evac_half(ps_oA, ps_oB, ouA, ouB, rs2, 0)
                        ps_oA = ops.tile([D + 1, 512], F32, tag="oA")
                        ps_oB = ops.tile([D + 1, 512], F32, tag="oB")
                    pending_o = (pt_t, jc, ps_oA, ps_oB, hA, hB)

                    # interleaved Q^T/K^T projections:
                    # pairs 0-4: Q(m+2) slots 0-7, K(m+2) slots 8-15 (2 MM/slot)
                    # pair 5: Q(7) spread 1 MM/slot; pair 6: K(7) 1 MM/slot
                    if m < NC - 3:
                        ch = m + 2
                        w_p, a_p, o_p = (wq_sb, bt_sb, qt) if s < 8 else (wk_sb, at_sb, kt)
                        e = s % 8
                        if pj is None:
                            pj = pjp.tile([128, T], F32, tag="pj")
                        for icc in range(2):
                            nc.tensor.matmul(
                                pj[:, bass.ts(icc, 512)],
                                w_p[:, e, bass.ts(ch, 128)],
                                a_p[:, e, bass.ts(icc, 512)],
                                start=(e == 0), stop=(e == NC - 1))
                        if s in (7, 15):
                            with nc.allow_low_precision(reason="QK fp16"):
                                nc.vector.tensor_copy(o_p[:, ch, :], pj[:])
                            pj = None
                    elif m in (NC - 3, NC - 2):
                        ch = NC - 1
                        w_p, a_p, o_p = ((wq_sb, bt_sb, qt) if m == NC - 3
                                         else (wk_sb, at_sb, kt))
                        e, icc = s // 2, s % 2
                        if pj is None:
                            pj = pjp.tile([128, T], F32, tag="pj")
                        nc.tensor.matmul(
                            pj[:, bass.ts(icc, 512)],
                            w_p[:, e, bass.ts(ch, 128)],
                            a_p[:, e, bass.ts(icc, 512)],
                            start=(e == 0), stop=(e == NC - 1))
                        if s == 15:
                            with nc.allow_low_precision(reason="QK fp16"):
                                nc.vector.tensor_copy(o_p[:, ch, :], pj[:])
                            pj = None

                # flush last O slot of this pair, then evacuate half 1
                issue_o(pending_o)
                pending_o = None
                evac_half(ps_oA, ps_oB, ouA, ouB, rs2, 1)
                pending_norm = (m, ouA, ouB, rs2)

            issue_norm(pending_norm)
            pending_norm = None

        if _KDBG:
            with tc.tile_pool(name="zdbg2", bufs=1) as zp2:
                zd = _DBG_TILES["zd"]
                nc.vector.tensor_copy(zd[0:64, 3, :], ot[0:64, 0, :])
                nc.vector.tensor_copy(zd[64:128, 3, :], ot[64:128, 0, :])
                nc.vector.tensor_copy(zd[:, 5, :], qt[:, 2, :])
                for cc in range(6):
                    nc.sync.dma_start(z_t[cc * 128:(cc + 1) * 128, :],
                                      zd[:, cc, :])

        if _KDBG2:
            with tc.tile_pool(name="zdbg3", bufs=2) as zp3:
                for mm in range(NC):
                    zc = zp3.tile([128, T], F32, tag="zc", name="zc")
                    nc.vector.tensor_copy(zc[:], ot[:, mm, :])
                    nc.sync.dma_start(z_t[mm * 128:(mm + 1) * 128, :], zc[:])

        # ---------------- Z: out-projection ----------------
        if not _KDBG and not _KDBG2:
          with tc.tile_pool(name="zps", bufs=2, space="PSUM") as zps, \
             tc.tile_pool(name="zsb", bufs=2) as zsbp:
            for cc in range(NC):
                ps = zps.tile([128, T], F32, tag="z")
                for mm in range(NC):
                    for ic in range(2):
                        nc.tensor.matmul(
                            ps[:, bass.ts(ic, 512)],
                            wo_sb[:, mm, bass.ts(cc, 128)],
                            ot[:, mm, bass.ts(ic, 512)],
                            start=(mm == 0), stop=(mm == NC - 1))
                zsb = zsbp.tile([128, T], F32, tag="zsb")
                nc.vector.tensor_copy(zsb[:], ps[:])
                nc.sync.dma_start(z_t[cc * 128:(cc + 1) * 128, :], zsb[:])
    nc.compile()
    return nc


def _group_w(wqkv, k):
    """Rows of Wqkv (3E, E) for q/k/v (k=0/1/2), grouped head-major.

    Row index layout: r = di*(3H) + k*H + h  ->  grouped[h*D+di, :].
    """
    w = np.asarray(wqkv, dtype=np.float32).reshape(D, 3, H, E)[:, k]   # [di, h, e]
    return np.ascontiguousarray(w.transpose(1, 0, 2).reshape(E, E))    # [h*D+di, e]


def kernel(x, y, Wqkv1, Wqkv2, Wout1, Wout2):
    x = np.asarray(x, dtype=np.float32)
    y = np.asarray(y, dtype=np.float32)

    if "nc" not in _NC_CACHE:
        _NC_CACHE["nc"] = _build()
    nc = _NC_CACHE["nc"]

    wq1_t = np.ascontiguousarray(_group_w(Wqkv1, 0).T)
    wk1_t = np.ascontiguousarray(_group_w(Wqkv1, 1).T)
    wv1_t = np.ascontiguousarray(_group_w(Wqkv1, 2).T)
    wq2_t = np.ascontiguousarray(_group_w(Wqkv2, 0).T)
    wk2_t = np.ascontiguousarray(_group_w(Wqkv2, 1).T)
    wv2_t = np.ascontiguousarray(_group_w(Wqkv2, 2).T)
    wout1_t = np.ascontiguousarray(np.asarray(Wout1, dtype=np.float32).T)
    wout2_t = np.ascontiguousarray(np.asarray(Wout2, dtype=np.float32).T)

    in_maps = []
    for c in range(N_CORES):
        s, b = divmod(c, B)
        if s == 0:
            # stream-1 output: K,V from x via Wqkv1; Q from y via Wqkv2
            a_t, b_t = x[b].T, y[b].T
            wq, wk, wv, wo = wq2_t, wk1_t, wv1_t, wout1_t
        else:
            a_t, b_t = y[b].T, x[b].T
            wq, wk, wv, wo = wq1_t, wk2_t, wv2_t, wout2_t
        in_maps.append({
            "a_t": np.ascontiguousarray(a_t).astype(np.float16),
            "b_t": np.ascontiguousarray(b_t).astype(np.float16),
            "wq_t": wq.astype(np.float16), "wk_t": wk.astype(np.float16),
            "wv_t": wv.astype(np.float16), "wout_t": wo.astype(np.float16),
        })

    trace = os.environ.get("BASS_KERNEL_TRACE", "0") == "1"
    if trace:
        try:
            from antenv.axon_hooks import get_axon_ntff_profile_hook  # noqa: F401
        except ImportError:
            trace = False
    ncores = int(os.environ.get("KCORES", str(N_CORES)))
    r = bass_utils.run_bass_kernel_spmd(nc, in_maps[:ncores], core_ids=list(range(ncores)),
                                        trace=trace)
    LAST_RESULTS["exec_time_ns"] = r.exec_time_ns
    LAST_RESULTS["profile_json"] = r.profile_json

    out1 = np.stack([r.results[b]["z_t"].T for b in range(B)]).astype(np.float32)
    out2 = np.stack([r.results[B + b]["z_t"].T for b in range(B)]).astype(np.float32)
    return out1, out2


# revision 27
# speedup vs baseline: 1.0531x; 1.0303x over previous
"""Trainium2 Bass kernel for nn_MultiHeadCrossAttention (B=4, T=1024, E=1024, H=16).

Sharding: 8 fully independent shards, zero cross-core communication:
(output stream s, batch b) for s in {1,2}, b in 0..3. Stream-1 output
xo@Wout1 needs K,V from x and Q from y; stream-2 the reverse.

Per-core kernel (activations transposed, feature-on-partition):
  Preamble: V = A^T.T @ Wv^T (natural, with ones column per head for the
  rowsum trick); Q^T/K^T chunks 0,1.
  m-loop over 8 head pairs (hA=2m, hB=2m+1), 16 slots (jc, ic) each:
    S^T pair via two concurrent K=64 matmuls (tile_position row split)
    P = exp(S/8) on ACT into SBUF fp16 (A|B merged per slot)
    O'^T accumulation (M=65 incl. ones row -> rowsum) lagged one slot
    Q^T/K^T projection chunk m+2 interleaved (2 matmuls per slot)
    normalization of pair m-1 lagged: reciprocal_approx_fast + gpsimd
    partition_broadcast + fp16 multiply into ot
  Tail: Z^T = Wout^T.T @ O^T accumulated over head pairs, DMA out.
"""

import os
import sys

sys.path.insert(0, "/opt/trn_rl_repo")

import numpy as np
import ml_dtypes
from contextlib import ExitStack

import concourse.bass as bass
import concourse.mybir as mybir
import concourse.tile as tile
from concourse import bacc
from concourse import bass_utils

B, T, E, H = 4, 1024, 1024, 16
D = E // H            # 64
NC = E // 128         # 8 chunks of 128
N_CORES = 8

F32 = mybir.dt.float32
F16 = mybir.dt.float16

_NC_CACHE = {}
LAST_RESULTS = {}
_KDBG = os.environ.get("KDBG", "0") == "1"
_KDBG2 = os.environ.get("KDBG", "0") == "2"
_DBG_TILES = {}


def _build():
    nc = bacc.Bacc("TRN2", target_bir_lowering=False, debug=False,
                   enable_asserts=False, num_devices=N_CORES)
    a_t = nc.dram_tensor("a_t", (E, T), F16, kind="ExternalInput").ap()
    b_t = nc.dram_tensor("b_t", (E, T), F16, kind="ExternalInput").ap()
    wq_t = nc.dram_tensor("wq_t", (E, E), F16, kind="ExternalInput").ap()
    wk_t = nc.dram_tensor("wk_t", (E, E), F16, kind="ExternalInput").ap()
    wv_t = nc.dram_tensor("wv_t", (E, E), F16, kind="ExternalInput").ap()
    wout_t = nc.dram_tensor("wout_t", (E, E), F16, kind="ExternalInput").ap()
    z_t = nc.dram_tensor("z_t", (E, T), F32, kind="ExternalOutput").ap()

    EXP = mybir.ActivationFunctionType.Exp

    with tile.TileContext(nc) as tc, ExitStack() as ctx:
        persist = ctx.enter_context(tc.tile_pool(name="persist", bufs=1))
        qt = persist.tile([128, NC, T], F16, tag="qt")
        kt = persist.tile([128, NC, T], F16, tag="kt")
        v = persist.tile([128, NC, H * (D + 1)], F16, tag="v")
        ot = persist.tile([128, NC, T], F16, tag="ot")
        wo_sb = None if _KDBG else persist.tile([128, NC, E], F16, tag="wo", name="wo_sb")

        for mch in range(NC):
            nc.vector.memset(
                v[:, mch, :].rearrange("p (h x) -> p h x", x=D + 1)[:, :, D:D + 1],
                1.0)
        if _KDBG:
            _DBG_TILES["zd"] = persist.tile([128, 6, T], F32, tag="zd", name="zd")
            nc.vector.memset(_DBG_TILES["zd"][:], 0.0)

        acts = ctx.enter_context(tc.tile_pool(name="acts", bufs=1))
        at_sb = acts.tile([128, NC, T], F16, tag="at")
        bt_sb = acts.tile([128, NC, T], F16, tag="bt")
        wq_sb = acts.tile([128, NC, E], F16, tag="wq")
        wk_sb = acts.tile([128, NC, E], F16, tag="wk")

        # ---------------- Preamble: V proj + QK chunks 0,1 ----------------
        with tc.tile_pool(name="wvp", bufs=1) as wvp, \
             tc.tile_pool(name="pps", bufs=3, space="PSUM") as pps:
            wv_sb = wvp.tile([128, NC, E], F16, tag="wv")
            # DMA priority: (at, wv) pairs first so V proj starts ASAP
            for e in range(NC):
                nc.sync.dma_start(at_sb[:, e, :], a_t[e * 128:(e + 1) * 128, :])
                nc.sync.dma_start(wv_sb[:, e, :], wv_t[e * 128:(e + 1) * 128, :])
            for e in range(NC):
                nc.sync.dma_start(bt_sb[:, e, :], b_t[e * 128:(e + 1) * 128, :])
                nc.sync.dma_start(wq_sb[:, e, :], wq_t[e * 128:(e + 1) * 128, :])
            for e in range(NC):
                nc.sync.dma_start(wk_sb[:, e, :], wk_t[e * 128:(e + 1) * 128, :])
            for e in range(NC) if not _KDBG else ():
                nc.sync.dma_start(wo_sb[:, e, :], wout_t[e * 128:(e + 1) * 128, :])

            # V natural: out[j-chunk m][h*d] = sum_e at[e, j].T @ wv[e, hd]
            for mch in range(NC):
                ps = pps.tile([128, T], F32, tag="pp")
                for e in range(NC):
                    for ic in range(2):
                        nc.tensor.matmul(
                            ps[:, bass.ts(ic, 512)],
                            at_sb[:, e, bass.ts(mch, 128)],
                            wv_sb[:, e, bass.ts(ic, 512)],
                            start=(e == 0), stop=(e == NC - 1))
                with nc.allow_low_precision(reason="V fp16 feeds fp16 matmul"):
                    nc.vector.tensor_copy(
                        v[:, mch, :].rearrange("p (h x) -> p h x", x=D + 1)[:, :, 0:D],
                        ps[:].rearrange("p (h d) -> p h d", d=D))

            # Q^T/K^T chunks 0 and 1
            for ch in (0, 1):
                for (w_sb, act_sb, out_sb) in ((wq_sb, bt_sb, qt), (wk_sb, at_sb, kt)):
                    ps = pps.tile([128, T], F32, tag="pp")
                    for e in range(NC):
                        for ic in range(2):
                            nc.tensor.matmul(
                                ps[:, bass.ts(ic, 512)],
                                w_sb[:, e, bass.ts(ch, 128)],
                                act_sb[:, e, bass.ts(ic, 512)],
                                start=(e == 0), stop=(e == NC - 1))
                    with nc.allow_low_precision(reason="QK fp16 feeds fp16 matmul"):
                        nc.vector.tensor_copy(out_sb[:, ch, :], ps[:])

        # ---------------- m-loop: attention over 8 head pairs ----------------
        with tc.tile_pool(name="sps", bufs=2, space="PSUM") as sps_pool, \
             tc.tile_pool(name="ops", bufs=1, space="PSUM") as ops, \
             tc.tile_pool(name="pjp", bufs=1, space="PSUM") as pjp, \
             tc.tile_pool(name="ptp", bufs=8) as ptp, \
             tc.tile_pool(name="oup", bufs=2) as oup, \
             tc.tile_pool(name="nrm", bufs=2) as nrm:

            SLOTS = [(ic, jc) for ic in range(2) for jc in range(NC)]
            pending_o = None   # (pt_tile, jc, ic, ps_oA, ps_oB, hA, hB)
            pending_norm = None  # (m, ouA, ouB, rs)

            def issue_o(po):
                pt_prev, jc, psA, psB, hA, hB = po
                st = dict(start=(jc == 0), stop=(jc == NC - 1))
                nc.tensor.matmul(psA[:, :],
                                 v[:, jc, hA * (D + 1):(hA + 1) * (D + 1)],
                                 pt_prev[:, 0:512], **st)
                nc.tensor.matmul(psB[:, :],
                                 v[:, jc, hB * (D + 1):(hB + 1) * (D + 1)],
                                 pt_prev[:, 512:1024], **st)

            def issue_norm(pn, lo=0, hi=T):
                mm, ouA, ouB, rs2 = pn
                w = hi - lo
                rr2 = nrm.tile([1, 2, T], F32, tag="rr2", bufs=1, name="rr2")
                nc.vector.reciprocal_approx_fast(rr2[:, :, lo:hi], rs2[:, :, lo:hi])
                rrh2 = nrm.tile([1, 2, T], F16, tag="rrh2", bufs=1, name="rrh2")
                with nc.allow_low_precision(reason="recip feeds fp16 multiply"):
                    nc.vector.tensor_copy(rrh2[:, :, lo:hi], rr2[:, :, lo:hi])
                bcA = nrm.tile([64, T], F16, tag="bcA", bufs=1, name="bcA")
                bcB = nrm.tile([64, T], F16, tag="bcB", bufs=1, name="bcB")
                nc.gpsimd.partition_broadcast(bcA[:, lo:hi], rrh2[:, 0, lo:hi])
                nc.gpsimd.partition_broadcast(bcB[:, lo:hi], rrh2[:, 1, lo:hi])
                with nc.allow_low_precision(reason="O^T fp16 feeds fp16 out-proj"):
                    nc.vector.tensor_mul(ot[0:64, mm, lo:hi], ouA[:, lo:hi], bcA[:, lo:hi])
                    nc.vector.tensor_mul(ot[64:128, mm, lo:hi], ouB[:, lo:hi], bcB[:, lo:hi])
                if _KDBG and mm == 0:
                    zd = _DBG_TILES["zd"]
                    nc.vector.tensor_copy(zd[0:64, 0, :], ouB[:])
                    nc.vector.tensor_copy(zd[0:64, 1, :], bcB[:])
                    nc.vector.tensor_copy(zd[0:1, 2, :], rs2[:, 1, :])
                    nc.vector.tensor_copy(zd[32:33, 2, :], rr2[:, 1, :])
                    nc.vector.tensor_copy(zd[64:65, 2, :], rrh2[:, 1, :])

            def evac_half(psA, psB, ouA, ouB, rs2, ic):
                sl = bass.ts(ic, 512)
                with nc.allow_low_precision(reason="O' fp16 feeds fp16 multiply"):
                    nc.vector.tensor_copy(ouA[:, sl], psA[0:D, :])
                    nc.vector.tensor_copy(ouB[:, sl], psB[0:D, :])
                nc.vector.tensor_copy(rs2[:, 0, sl], psA[D:D + 1, :])
                nc.vector.tensor_copy(rs2[:, 1, sl], psB[D:D + 1, :])

            for m in range(NC):
                hA, hB = 2 * m, 2 * m + 1
                ps_oA = ops.tile([D + 1, 512], F32, tag="oA")
                ps_oB = ops.tile([D + 1, 512], F32, tag="oB")
                ouA = oup.tile([D, T], F16, tag="ouA")
                ouB = oup.tile([D, T], F16, tag="ouB")
                rs2 = nrm.tile([1, 2, T], F32, tag="rs2", bufs=1)
                if pending_norm is not None:
                    issue_norm(pending_norm)
                    pending_norm = None

                pj = None
                def issue_s(ic, jc):
                    sps = sps_pool.tile([128, 1024], F32, tag="s", name="sps")
                    nc.tensor.matmul(
                        sps[:, 0:512],
                        kt[0:64, m, bass.ts(jc, 128)],
                        qt[0:64, m, bass.ts(ic, 512)],
                        start=True, stop=True)
                    nc.tensor.matmul(
                        sps[:, 512:1024],
                        kt[64:128, m, bass.ts(jc, 128)],
                        qt[64:128, m, bass.ts(ic, 512)],
                        start=True, stop=True, tile_position=(64, 0))
                    return sps

                pending_s = issue_s(*SLOTS[0])
                for s, (ic, jc) in enumerate(SLOTS):
                    if s + 1 < len(SLOTS):
                        next_s = issue_s(*SLOTS[s + 1])
                    sps, pending_s = pending_s, next_s if s + 1 < len(SLOTS) else None
                    pt_t = ptp.tile([128, 1024], F16, tag="pt")
                    nc.scalar.activation(pt_t[:], sps[:], EXP, scale=0.125)
                    if _KDBG and m == 0 and s == 0:
                        nc.vector.tensor_copy(_DBG_TILES["zd"][:, 4, :], pt_t[:])

                    if pending_o is not None:
                        issue_o(pending_o)
                    if s == 8:
                        evac_half(ps_oA, ps_oB, ouA, ouB, rs2, 0)
                        if m == NC - 1:
                            issue_norm((m, ouA, ouB, rs2), 0, 512)
                        ps_oA = ops.tile([D + 1, 512], F32, tag="oA")
                        ps_oB = ops.tile([D + 1, 512], F32, tag="oB")
                    pending_o = (pt_t, jc, ps_oA, ps_oB, hA, hB)

                    # interleaved Q^T/K^T projections:
                    # pairs 0-4: Q(m+2) slots 0-7, K(m+2) slots 8-15 (2 MM/slot)
                    # pair 5: Q(7) spread 1 MM/slot; pair 6: K(7) 1 MM/slot
                    if m < NC - 3:
                        ch = m + 2
                        w_p, a_p, o_p = (wq_sb, bt_sb, qt) if s < 8 else (wk_sb, at_sb, kt)
                        e = s % 8
                        if pj is None:
                            pj = pjp.tile([128, T], F32, tag="pj")
                        for icc in range(2):
                            nc.tensor.matmul(
                                pj[:, bass.ts(icc, 512)],
                                w_p[:, e, bass.ts(ch, 128)],
                                a_p[:, e, bass.ts(icc, 512)],
                                start=(e == 0), stop=(e == NC - 1))
                        if s in (7, 15):
                            with nc.allow_low_precision(reason="QK fp16"):
                                nc.vector.tensor_copy(o_p[:, ch, :], pj[:])
                            pj = None
                    elif m in (NC - 3, NC - 2):
                        ch = NC - 1
                        w_p, a_p, o_p = ((wq_sb, bt_sb, qt) if m == NC - 3
                                         else (wk_sb, at_sb, kt))
                        e, icc = s // 2, s % 2
                        if pj is None:
                            pj = pjp.tile([128, T], F32, tag="pj")
                        nc.tensor.matmul(
                            pj[:, bass.ts(icc, 512)],
                            w_p[:, e, bass.ts(ch, 128)],
                            a_p[:, e, bass.ts(icc, 512)],
                            start=(e == 0), stop=(e == NC - 1))
                        if s == 15:
                            with nc.allow_low_precision(reason="QK fp16"):
                                nc.vector.tensor_copy(o_p[:, ch, :], pj[:])
                            pj = None

                # flush last O slot of this pair, then evacuate half 1
                issue_o(pending_o)
                pending_o = None
                evac_half(ps_oA, ps_oB, ouA, ouB, rs2, 1)
                pending_norm = (m, ouA, ouB, rs2)

            issue_norm(pending_norm, 512, T)
            pending_norm = None

        if _KDBG:
            with tc.tile_pool(name="zdbg2", bufs=1) as zp2:
                zd = _DBG_TILES["zd"]
                nc.vector.tensor_copy(zd[0:64, 3, :], ot[0:64, 0, :])
                nc.vector.tensor_copy(zd[64:128, 3, :], ot[64:128, 0, :])
                nc.vector.tensor_copy(zd[:, 5, :], qt[:, 2, :])
                for cc in range(6):
                    nc.sync.dma_start(z_t[cc * 128:(cc + 1) * 128, :],
                                      zd[:, cc, :])

        if _KDBG2:
            with tc.tile_pool(name="zdbg3", bufs=2) as zp3:
                for mm in range(NC):
                    zc = zp3.tile([128, T], F32, tag="zc", name="zc")
                    nc.vector.tensor_copy(zc[:], ot[:, mm, :])
                    nc.sync.dma_start(z_t[mm * 128:(mm + 1) * 128, :], zc[:])

        # ---------------- Z: out-projection ----------------
        if not _KDBG and not _KDBG2:
          with tc.tile_pool(name="zps", bufs=2, space="PSUM") as zps, \
             tc.tile_pool(name="zsb", bufs=2) as zsbp:
            for cc in range(NC):
                ps = zps.tile([128, T], F32, tag="z")
                for mm in range(NC):
                    for ic in range(2):
                        nc.tensor.matmul(
                            ps[:, bass.ts(ic, 512)],
                            wo_sb[:, mm, bass.ts(cc, 128)],
                            ot[:, mm, bass.ts(ic, 512)],
                            start=(mm == 0), stop=(mm == NC - 1))
                zsb = zsbp.tile([128, T], F32, tag="zsb")
                nc.vector.tensor_copy(zsb[:], ps[:])
                nc.sync.dma_start(z_t[cc * 128:(cc + 1) * 128, :], zsb[:])
    nc.compile()
    return nc


def _group_w(wqkv, k):
    """Rows of Wqkv (3E, E) for q/k/v (k=0/1/2), grouped head-major.

    Row index layout: r = di*(3H) + k*H + h  ->  grouped[h*D+di, :].
    """
    w = np.asarray(wqkv, dtype=np.float32).reshape(D, 3, H, E)[:, k]   # [di, h, e]
    return np.ascontiguousarray(w.transpose(1, 0, 2).reshape(E, E))    # [h*D+di, e]


def kernel(x, y, Wqkv1, Wqkv2, Wout1, Wout2):
    x = np.asarray(x, dtype=np.float32)
    y = np.asarray(y, dtype=np.float32)

    if "nc" not in _NC_CACHE:
        _NC_CACHE["nc"] = _build()
    nc = _NC_CACHE["nc"]

    wq1_t = np.ascontiguousarray(_group_w(Wqkv1, 0).T)
    wk1_t = np.ascontiguousarray(_group_w(Wqkv1, 1).T)
    wv1_t = np.ascontiguousarray(_group_w(Wqkv1, 2).T)
    wq2_t = np.ascontiguousarray(_group_w(Wqkv2, 0).T)
    wk2_t = np.ascontiguousarray(_group_w(Wqkv2, 1).T)
    wv2_t = np.ascontiguousarray(_group_w(Wqkv2, 2).T)
    wout1_t = np.ascontiguousarray(np.asarray(Wout1, dtype=np.float32).T)
    wout2_t = np.ascontiguousarray(np.asarray(Wout2, dtype=np.float32).T)

    in_maps = []
    for c in range(N_CORES):
        s, b = divmod(c, B)
        if s == 0:
            # stream-1 output: K,V from x via Wqkv1; Q from y via Wqkv2
            a_t, b_t = x[b].T, y[b].T
            wq, wk, wv, wo = wq2_t, wk1_t, wv1_t, wout1_t
        else:
            a_t, b_t = y[b].T, x[b].T
            wq, wk, wv, wo = wq1_t, wk2_t, wv2_t, wout2_t
        in_maps.append({
            "a_t": np.ascontiguousarray(a_t).astype(np.float16),
            "b_t": np.ascontiguousarray(b_t).astype(np.float16),
            "wq_t": wq.astype(np.float16), "wk_t": wk.astype(np.float16),
            "wv_t": wv.astype(np.float16), "wout_t": wo.astype(np.float16),
        })

    trace = os.environ.get("BASS_KERNEL_TRACE", "0") == "1"
    if trace:
        try:
            from antenv.axon_hooks import get_axon_ntff_profile_hook  # noqa: F401
        except ImportError:
            trace = False
    ncores = int(os.environ.get("KCORES", str(N_CORES)))
    r = bass_utils.run_bass_kernel_spmd(nc, in_maps[:ncores], core_ids=list(range(ncores)),
                                        trace=trace)
    LAST_RESULTS["exec_time_ns"] = r.exec_time_ns
    LAST_RESULTS["profile_json"] = r.profile_json

    out1 = np.stack([r.results[b]["z_t"].T for b in range(B)]).astype(np.float32)
    out2 = np.stack([r.results[B + b]["z_t"].T for b in range(B)]).astype(np.float32)
    return out1, out2
